# revision 1
# baseline (speedup 1.0000x reference)
"""Trainium2 Bass kernel for DynamicAdjGenerator.

Per timestep t (fully independent): MHA self-attention over B=512 tokens
(D=768, H=12 heads), then pairwise cosine similarity of the outputs,
mean over the first index -> adj (B, 1).

Sharding: data-parallel over T=64 timesteps across 8 NeuronCores
(8 timesteps per core); small MHA params replicated.

All matmuls run on the PE in fp16 (inputs rounded fp16, fp32 PSUM
accumulation). The cosine stage avoids materializing the (B,B) Gram
matrix: adj_j = (1/B) * rn_j * (w . out_j) with w = sum_i rn_i out_i.
"""

import numpy as np

T, B, D, H = 64, 512, 768, 12
HD = D // H          # 64
NCORES = 8
TL = T // NCORES     # 8 timesteps per core
P = 128
KD = D // P          # 6 k-tiles over D
BQ = B // P          # 4 chunks over B
M_QK = (2 * D) // P  # 12 chunks of the stacked q,k rows
EPS = 1e-8

_CACHE = {}


def _build_nc(has_bias: bool, opts=None):
    opts = opts or {}
    import concourse.bass as bass
    import concourse.bacc as bacc
    import concourse.tile as tile
    import concourse.mybir as mybir
    from concourse.masks import make_identity

    f32 = mybir.dt.float32
    f16 = mybir.dt.float16
    f8 = mybir.dt.float8e4
    DR = mybir.MatmulPerfMode.DoubleRow
    AF = mybir.ActivationFunctionType
    ALU = mybir.AluOpType

    nc = bacc.Bacc()

    x_d = nc.dram_tensor("x", [TL, B, D], f32, kind="ExternalInput")
    wi_d = nc.dram_tensor("wi", [3 * D, D], f32, kind="ExternalInput")
    bi_d = nc.dram_tensor("bi", [3 * D], f32, kind="ExternalInput")
    wo_d = nc.dram_tensor("wo", [D, D], f32, kind="ExternalInput")
    bo_d = nc.dram_tensor("bo", [D], f32, kind="ExternalInput")
    adj_d = nc.dram_tensor("adj", [TL, B, 1], f32, kind="ExternalOutput")
    adj_flat = adj_d.rearrange("t b one -> t (b one)")

    with tile.TileContext(nc) as tc, \
         tc.tile_pool(name="consts", bufs=1) as consts, \
         tc.tile_pool(name="xin", bufs=2) as xin, \
         tc.tile_pool(name="xt", bufs=2) as xtp, \
         tc.tile_pool(name="qk", bufs=3) as qkp, \
         tc.tile_pool(name="vp", bufs=3) as vp, \
         tc.tile_pool(name="attn", bufs=2) as attnp, \
         tc.tile_pool(name="ctx", bufs=2) as ctxp, \
         tc.tile_pool(name="outp", bufs=2) as outp, \
         tc.tile_pool(name="vec", bufs=2) as vecp, \
         tc.tile_pool(name="ps", bufs=8, space="PSUM") as psp:

        def ps(shape=(P, 512), dtype=f32, kind="mm"):
            bufs = {"mm": 4, "sc": 2, "ctx": 2}[kind]
            return psp.tile(list(shape), dtype, tag=kind, name="ps",
                            bufs=bufs)

        # ---- constants ----
        ident = consts.tile([P, P], f32)
        make_identity(nc, ident)
        ones_r128 = consts.tile([1, P], f16)  # lhsT for partition-broadcasts
        nc.vector.memset(ones_r128, 1.0)
        ones_r512 = consts.tile([1, B], f16)  # rhs for bias broadcasts
        nc.vector.memset(ones_r512, 1.0)
        ones_col = consts.tile([P, 1], f16)   # lhsT for partition-dim sums
        nc.vector.memset(ones_col, 1.0)

        # ---- weights: load natural layout, PE-transpose to (d, m) fp16 ----
        # wqkT[p, kd, m] = Wi[m, kd*128+p], m in 0..2D-1 (q rows pre-scaled 1/8)
        # wvT [p, kd, n] = Wi[2D+n, kd*128+p]
        # woT [p, kd, e] = Wo[e, kd*128+p]
        wqkT = consts.tile([P, KD, 2 * D], f16)
        wvT = consts.tile([P, KD, D], f16)
        woT = consts.tile([P, KD, D], f16)
        qscale = 1.0 / np.sqrt(HD)

        def load_transposed(src_ap, n_chunks, dst_fn):
            # src rows (n_chunks*128, D) f32 -> dst[p, kd, chunk*128+128]
            for c4 in range((n_chunks + 3) // 4):
                cn = min(4, n_chunks - c4 * 4)
                wt = xin.tile([P, 4, D], f32, tag="xti", name="wt")
                nc.sync.dma_start(
                    out=wt[:, :cn, :],
                    in_=src_ap.rearrange("(c p) d -> p c d", p=P)[:, c4 * 4:c4 * 4 + cn, :],
                )
                for kd in range(KD):
                    pt = ps((P, 512))
                    for c in range(cn):
                        nc.tensor.transpose(
                            pt[:, c * P:(c + 1) * P],
                            wt[:, c, kd * P:(kd + 1) * P],
                            ident,
                        )
                    for c in range(cn):
                        dst, scale = dst_fn(c4 * 4 + c, kd)
                        if scale != 1.0:
                            nc.vector.tensor_scalar_mul(
                                dst, pt[:, c * P:(c + 1) * P], scale)
                        else:
                            nc.vector.tensor_copy(
                                out=dst, in_=pt[:, c * P:(c + 1) * P])

        if has_bias:
            # bqk (1, 2D) fp16, q part scaled; bv (1, D) fp16; bo (128, KD) f32
            brow = consts.tile([1, 3 * D], f32)
            nc.sync.dma_start(out=brow, in_=bi_d[None, :])
            bqk16 = consts.tile([1, 2 * D], f16)
            nc.vector.tensor_scalar_mul(bqk16[:, :D], brow[:, :D], qscale)
            nc.vector.tensor_copy(out=bqk16[:, D:], in_=brow[:, D:2 * D])
            bv16 = consts.tile([1, D], f16)
            nc.vector.tensor_copy(out=bv16, in_=brow[:, 2 * D:])
            bop = consts.tile([P, KD], f32)
            nc.sync.dma_start(
                out=bop, in_=bo_d.rearrange("(kd p) -> p kd", p=P))

        # ---- software-pipelined per-timestep schedule ----
        # Emission order shapes each engine's instruction stream. Per
        # iteration: attention(t) with scores(m)/ctx(m-1) pipelining,
        # then projections A(t+2) (fills the DVE/Pool normalization tail),
        # then out-proj + nsq chain of t, then the deferred cosine tail
        # u(t-1) whose scalar chain had a full iteration of slack.

        def load_x(t):
            """Load x_t natural layout, PE-transpose to xT (d, b) fp16."""
            ctx_scope = nc.named_scope(f"load_x_{t}"); ctx_scope.__enter__()
            xti = xin.tile([P, BQ, D], f32, tag="xti", name="xti")
            nc.gpsimd.dma_start(
                out=xti, in_=x_d[t].rearrange("(c p) d -> p c d", p=P))
            xT = xtp.tile([P, KD, B], f16, tag="xT", name="xT")
            for kd in range(KD):
                pt = ps((P, 512))
                for c in range(BQ):
                    nc.tensor.transpose(
                        pt[:, c * P:(c + 1) * P],
                        xti[:, c, kd * P:(kd + 1) * P],
                        ident,
                    )
                nc.vector.tensor_copy(out=xT[:, kd, :], in_=pt)
            ctx_scope.__exit__(None, None, None)
            return xT

        def proj_qk(t, xT):
            sc_ = nc.named_scope(f"proj_qk_{t}"); sc_.__enter__()
            # qkT (2D, B): chunk m holds heads 2m (parts 0-63), 2m+1 (64-127)
            qk = qkp.tile([P, M_QK, B], f16, tag="qk", name="qk")
            for m in range(M_QK):
                pt = ps()
                for kd in range(KD):
                    nc.tensor.matmul(
                        pt, lhsT=wqkT[:, kd, m * P:(m + 1) * P],
                        rhs=xT[:, kd, :], start=(kd == 0),
                        stop=(kd == KD - 1 and not has_bias))
                if has_bias:
                    nc.tensor.matmul(
                        pt, lhsT=bqk16[:, m * P:(m + 1) * P],
                        rhs=ones_r512, start=False, stop=True)
                if m < KD:
                    nc.vector.tensor_scalar_mul(qk[:, m, :], pt, float(qscale))
                else:
                    nc.vector.tensor_copy(out=qk[:, m, :], in_=pt)

            sc_.__exit__(None, None, None)
            return qk

        def proj_v(t, xT):
            sc_ = nc.named_scope(f"proj_v_{t}"); sc_.__enter__()
            # v (b, d) with per-head ones column
            v_sb = vp.tile([P, BQ, H, HD + 1], f16, tag="v", name="v_sb")
            nc.vector.memset(v_sb[:, :, :, HD], 1.0)
            for kb in range(BQ):
                for half in range(2):
                    n0 = half * (D // 2)
                    pt = ps((P, D // 2))
                    for kd in range(KD):
                        nc.tensor.matmul(
                            pt, lhsT=xT[:, kd, kb * P:(kb + 1) * P],
                            rhs=wvT[:, kd, n0:n0 + D // 2],
                            start=(kd == 0),
                            stop=(kd == KD - 1 and not has_bias))
                    if has_bias:
                        nc.tensor.matmul(
                            pt, lhsT=ones_r128,
                            rhs=bv16[:, n0:n0 + D // 2],
                            start=False, stop=True)
                    h0 = half * (H // 2)
                    nc.vector.tensor_copy(
                        out=v_sb[:, kb, h0:h0 + H // 2, :HD],
                        in_=pt.rearrange("p (h d) -> p h d", d=HD))
            sc_.__exit__(None, None, None)
            return v_sb

        def phase_a(t):
            xT = load_x(t)
            return proj_qk(t, xT), proj_v(t, xT)

        def emit_scores(qk, m, t=-1):
            sc_ = nc.named_scope(f"scores_{t}_{m}"); sc_.__enter__()
            """Row-packed K=64 scoresT for head pair (2m, 2m+1) + exp."""
            attn = attnp.tile([P, 2, BQ, B], f16, tag="attn", name="attn")
            for kb in range(BQ):
                for hi in range(2):
                    pt = ps(kind="sc")
                    nc.tensor.matmul(
                        pt,
                        lhsT=qk[hi * HD:(hi + 1) * HD, KD + m,
                                kb * P:(kb + 1) * P],
                        rhs=qk[hi * HD:(hi + 1) * HD, m, :],
                        start=True, stop=True,
                        tile_position=(hi * HD, 0))
                    nc.scalar.activation(attn[:, hi, kb, :], pt, AF.Exp)
            sc_.__exit__(None, None, None)
            return attn

        def emit_ctx(v_sb, attn, ctx, m, t=-1):
            sc_ = nc.named_scope(f"ctx_{t}_{m}"); sc_.__enter__()
            """ctxT for pair m from exp'd scores, with softmax norm."""
            for hi in range(2):
                h = 2 * m + hi
                pc = ps((HD + 1, 512), kind="ctx")
                for kb in range(BQ):
                    nc.tensor.matmul(
                        pc, lhsT=v_sb[:, kb, h, :],
                        rhs=attn[:, hi, kb, :],
                        start=(kb == 0), stop=(kb == BQ - 1))
                rs16 = vecp.tile([1, B], f16, tag="rs16", name="rs16")
                with nc.allow_low_precision("softmax normalizer"):
                    nc.vector.reciprocal(rs16, pc[HD:HD + 1, :])
                rb = vecp.tile([HD, B], f16, tag="rb", name="rb")
                nc.gpsimd.partition_broadcast(rb, rs16)
                nc.vector.tensor_tensor(
                    out=ctx[hi * HD:(hi + 1) * HD, m, :],
                    in0=pc[:HD, :], in1=rb, op=ALU.mult)
            sc_.__exit__(None, None, None)

        def phase_out(t, ctx):
            """Out-projection, squares, nsq + rn/w scalar chain."""
            sc_ = nc.named_scope(f"out_{t}"); sc_.__enter__()
            outT = outp.tile([P, KD, B], f16, tag="outT", name="outT")
            sq = outp.tile([P, KD, B], f16, tag="sq", name="sq")
            for e in range(KD):
                pt = ps()
                for kd in range(KD):
                    nc.tensor.matmul(
                        pt, lhsT=woT[:, kd, e * P:(e + 1) * P],
                        rhs=ctx[:, kd, :], start=(kd == 0),
                        stop=(kd == KD - 1))
                if has_bias:
                    nc.scalar.activation(
                        outT[:, e, :], pt, AF.Identity,
                        bias=bop[:, e:e + 1])
                else:
                    nc.scalar.copy(out=outT[:, e, :], in_=pt)
                nc.vector.tensor_mul(
                    out=sq[:, e, :], in0=outT[:, e, :], in1=outT[:, e, :])

            pn = ps((1, 512))
            for kd in range(KD):
                nc.tensor.matmul(
                    pn, lhsT=ones_col, rhs=sq[:, kd, :],
                    start=(kd == 0), stop=(kd == KD - 1))
            nsq = vecp.tile([1, B], f32, tag="nsq", name="nsq")
            nc.vector.tensor_scalar_max(nsq, pn, EPS * EPS)
            r2 = vecp.tile([1, B], f32, tag="r2", name="r2")
            nc.vector.reciprocal(r2, nsq)
            rn = vecp.tile([1, B], f32, tag="rn", name="rn")
            nc.scalar.activation(rn, r2, AF.Sqrt)
            rnb = vecp.tile([1, B], f16, tag="rnb", name="rnb")
            nc.scalar.activation(rnb, r2, AF.Sqrt, scale=1.0 / (B * B))

            rnbb = outp.tile([P, B], f16, tag="rnbb", name="rnbb")
            nc.gpsimd.partition_broadcast(rnbb, rnb)
            wsum = outp.tile([P, KD], f32, tag="wsum", name="wsum")
            wb = outp.tile([P, B], f16, tag="wb", name="wb")
            for kd in range(KD):
                nc.vector.tensor_tensor(
                    out=wb, in0=outT[:, kd, :], in1=rnbb, op=ALU.mult)
                nc.vector.reduce_sum(
                    wsum[:, kd:kd + 1], wb, axis=mybir.AxisListType.X)
            w16 = outp.tile([P, KD], f16, tag="w16", name="w16")
            nc.vector.tensor_copy(out=w16, in_=wsum)
            sc_.__exit__(None, None, None)
            return outT, rn, w16

        def phase_u(t, outT, rn, w16):
            """Deferred cosine tail: u = w^T outT, adj = u * rn."""
            sc_ = nc.named_scope(f"u_{t}"); sc_.__enter__()
            pu = ps((1, 512))
            for kd in range(KD):
                nc.tensor.matmul(
                    pu, lhsT=w16[:, kd:kd + 1], rhs=outT[:, kd, :],
                    start=(kd == 0), stop=(kd == KD - 1))
            adj = vecp.tile([1, B], f32, tag="adj", name="adj")
            nc.vector.tensor_tensor(out=adj, in0=pu, in1=rn, op=ALU.mult)
            nc.sync.dma_start(out=adj_flat[t:t + 1, :], in_=adj)
            sc_.__exit__(None, None, None)

        xT0 = load_x(0)
        xT1 = load_x(1)
        load_transposed(
            wi_d[:2 * D], M_QK,
            lambda m, kd: (wqkT[:, kd, m * P:(m + 1) * P], 1.0))
        qk0 = proj_qk(0, xT0)
        load_transposed(
            wi_d[2 * D:], KD,
            lambda m, kd: (wvT[:, kd, m * P:(m + 1) * P], 1.0))
        v0 = proj_v(0, xT0)
        load_transposed(
            wo_d, KD,
            lambda m, kd: (woT[:, kd, m * P:(m + 1) * P], 1.0))
        a_state = {0: (qk0, v0)}
        if TL > 1:
            a_state[1] = (proj_qk(1, xT1), proj_v(1, xT1))

        def phase_attn(t, qk, v_sb):
            ctx = ctxp.tile([P, KD, B], f16, tag="ctx", name="ctx")
            prev = None
            for m in range(KD):
                attn = emit_scores(qk, m, t)
                if prev is not None:
                    emit_ctx(v_sb, prev[0], ctx, prev[1], t)
                prev = (attn, m)
            emit_ctx(v_sb, prev[0], ctx, prev[1], t)
            return ctx

        tail = None
        for t in range(TL):
            ctx = phase_attn(t, *a_state.pop(t))
            if t + 2 < TL:
                a_state[t + 2] = phase_a(t + 2)
            if tail is not None:
                phase_u(t - 1, *tail)
            out_state = phase_out(t, ctx)
            tail = out_state
        phase_u(TL - 1, *tail)

    nc.compile()
    return nc


def kernel(node_embs, in_proj_w, in_proj_b, out_proj_w, out_proj_b):
    from concourse.bass_utils import run_bass_kernel_spmd

    node_embs = np.ascontiguousarray(node_embs, dtype=np.float32)
    in_proj_w = np.ascontiguousarray(in_proj_w, dtype=np.float32)
    in_proj_b = np.ascontiguousarray(in_proj_b, dtype=np.float32)
    out_proj_w = np.ascontiguousarray(out_proj_w, dtype=np.float32)
    out_proj_b = np.ascontiguousarray(out_proj_b, dtype=np.float32)

    has_bias = bool(np.any(in_proj_b) or np.any(out_proj_b))
    key = ("nc", has_bias)
    if key not in _CACHE:
        _CACHE[key] = _build_nc(has_bias)
    nc = _CACHE[key]

    in_maps = []
    for c in range(NCORES):
        in_maps.append({
            "x": node_embs[c * TL:(c + 1) * TL],
            "wi": in_proj_w,
            "bi": in_proj_b,
            "wo": out_proj_w,
            "bo": out_proj_b,
        })
    res = run_bass_kernel_spmd(nc, in_maps, list(range(NCORES)))
    out = np.concatenate([res.results[c]["adj"] for c in range(NCORES)], axis=0)
    return out.astype(np.float32)

